# revision 62
# baseline (speedup 1.0000x reference)
"""Multi-head causal attention with RoPE on 8 TRN2 NeuronCores.

Problem: B=2, T=2048, D=1024, H=16 heads, head_dim=64.
  out = softmax(mask(rope(x@Wq.T) @ rope(x@Wk.T).T / 8)) @ (x@Wv.T) @ Wo.T

Sharding: tensor-parallel over heads. Core c owns heads {2c, 2c+1} and
computes Q/K/V + attention for them over all 4096 tokens; four small
AllToAlls (one per (batch, token-half)) redistribute attention outputs
to token-sharded layout; row-parallel Wo with K=128 contraction tiles.
Core c's output rows are the interleaved 128-token tiles
{t : (t//128) % 8 == c} of each batch (re-assembled on the host).

Everything runs in bf16 (fp32 PSUM accumulation): same 1 cycle/row on
the PE as f32r but half the HBM/SBUF traffic and half the DVE cost.
RoPE uses a block-diagonal rotation matmul (engine ops cannot cross
partitions) + cos/sin elementwise on DVE. K stays 128 for every matmul
(scores use per-head zero-padded rope(K)); diagonal score tiles are
column-sliced to skip fully-masked query blocks. Softmax normalization
is batched per (batch, half): rowsums are staged by ScalarE on their
own partition, DMA'd to partition 0 in one batch, inverted as
exp(-ln(s)) on ScalarE, partition-broadcast on GpSimd, applied by DVE.
"""
import sys

sys.path.insert(0, "/opt/trn_rl_repo")

import numpy as np

from concourse import bacc, mybir, tile
from concourse import bass_utils

N_CORES = 8
B, T, D, H = 2, 2048, 1024, 16
HD = D // H              # 64
HPC = H // N_CORES       # 2 heads per core
BT = B * T               # 4096
NF = D // 128            # 8 feature chunks
NTC = BT // 512          # 8 t-chunks of 512
QCHUNK = 512

F32 = mybir.dt.float32
BF16 = mybir.dt.bfloat16

_CACHE = {}


def _rot_matrix():
    """R2 = blockdiag(R, R), R@u = rotate_half(u) per 64-dim head."""
    half = HD // 2
    R = np.zeros((HD, HD), dtype=np.float32)
    for i in range(half):
        R[i, i + half] = -1.0
        R[i + half, i] = 1.0
    R2 = np.zeros((2 * HD, 2 * HD), dtype=np.float32)
    R2[:HD, :HD] = R
    R2[HD:, HD:] = R
    return R2


def build(debug=False):
    nc = bacc.Bacc("TRN2", target_bir_lowering=False, debug=False,
                   num_devices=N_CORES)

    # ---- DRAM parameters (per-core shards, host-prepped layouts) ----
    xt = nc.declare_dram_parameter("xt", [NTC, 128, NF, 512], BF16, isOutput=False)
    wq_t = nc.declare_dram_parameter("wq_t", [128, NF, 128], BF16, isOutput=False)
    wk_t = nc.declare_dram_parameter("wk_t", [128, NF, 128], BF16, isOutput=False)
    wv_t = nc.declare_dram_parameter("wv_t", [128, NF, 128], BF16, isOutput=False)
    wo_t = nc.declare_dram_parameter("wo_t", [128, NF, D], BF16, isOutput=False)
    cos2 = nc.declare_dram_parameter("cos2", [128, T], BF16, isOutput=False)
    sin2 = nc.declare_dram_parameter("sin2", [128, T], BF16, isOutput=False)
    rot2t = nc.declare_dram_parameter("rot2t", [128, 128], BF16, isOutput=False)
    ident = nc.declare_dram_parameter("ident", [128, 128], BF16, isOutput=False)
    trimask = nc.declare_dram_parameter("trimask", [128, 128], BF16, isOutput=False)
    out = nc.declare_dram_parameter("out", [512, D], F32, isOutput=True)
    if debug:
        dbg_qrope = nc.declare_dram_parameter("dbg_qrope", [128, BT], F32, isOutput=True)
        dbg_krope = nc.declare_dram_parameter("dbg_krope", [128, HPC * BT], F32, isOutput=True)
        dbg_vall = nc.declare_dram_parameter("dbg_vall", [128, B * HPC * 16 * 65], F32, isOutput=True)
        dbg_attout = nc.declare_dram_parameter("dbg_attout", [64, B * HPC * T], F32, isOutput=True)
        dbg_attall = nc.declare_dram_parameter("dbg_attall", [128, B * N_CORES * 256], F32, isOutput=True)

    with tile.TileContext(nc) as tc, nc.allow_low_precision(reason="bf16 compute"):
        with (
            tc.tile_pool(name="consts", bufs=1) as cpool,
            tc.tile_pool(name="work", bufs=1) as wpool,
            tc.tile_pool(name="psum", bufs=1, space="PSUM") as ppool,
            tc.tile_pool(name="dram", bufs=1, space="DRAM") as dpool,
        ):
            # ---- persistent tensors ----
            rot_sb = cpool.tile([128, 128], BF16, tag="rot")
            id_sb = cpool.tile([128, 128], BF16, tag="ident")
            tri_sb = cpool.tile([128, 128], BF16, tag="tri")
            cos_sb = cpool.tile([128, T], BF16, tag="cos")
            sin_sb = cpool.tile([128, T], BF16, tag="sin")
            wo_sb = cpool.tile([128, NF, D], BF16, tag="wo")
            qrope = cpool.tile([128, BT], BF16, tag="qrope")
            # K rope, zero-padded per head so scores matmuls keep K=128.
            krope = cpool.tile([128, HPC, BT], BF16, tag="krope")
            # V per (b, h): [128 t-part, 16 t-tiles, 128]; cols 64:128 are
            # all-ones so the attv matmul emits the softmax rowsum
            # replicated across PSUM partitions 64:128 for free
            v_all = cpool.tile([128, B, HPC, T // 128, 128], BF16, tag="v_all")
            # attention out: head on the free dim (engine ops can't move
            # partitions); the A2A staging DMA interleaves the two heads
            # into the payload's 128 partitions
            att_out = cpool.tile([64, B, HPC, T], BF16, tag="att_out")
            # post-A2A: all 16 heads (rows (src, 2-head pair)) x my tokens;
            # one tile per (b, half) so proj(b, s) can't pick up a false
            # dependency on a later A2A's landing DMA
            att_alls = [[cpool.tile([128, N_CORES, 128], BF16,
                                    tag=f"att_all{b}{hf}",
                                    name=f"att_all{b}{hf}")
                         for hf in range(2)] for b in range(B)]

            wq_sb = cpool.tile([128, NF, 128], BF16, tag="wq")
            wk_sb = cpool.tile([128, NF, 128], BF16, tag="wk")
            wv_sb = cpool.tile([128, NF, 128], BF16, tag="wv")



            a2a_in = [[dpool.tile([N_CORES, 128, 128], BF16,
                                  tag=f"a2a_in{b}{hf}", name=f"a2a_in{b}{hf}")
                       for hf in range(2)] for b in range(B)]
            a2a_out = [[dpool.tile([N_CORES, 128, 128], BF16,
                                   tag=f"a2a_out{b}{hf}", name=f"a2a_out{b}{hf}")
                        for hf in range(2)] for b in range(B)]

            # ---- DMA loads; the first projection matmul needs only wq
            # f-chunk 0 + xt chunk-0 f-chunk 0, so those go first.
            nc.sync.dma_start(wq_sb[:, 0:1, :], wq_t[:, 0:1, :])

            def load_xt_half(j, half, nsplit=1, eng=None):
                """One 512-token, 4-feature-chunk half of x^T (contiguous)."""
                xh = wpool.tile([128, NF // 2, 512], BF16, tag="xt", bufs=3,
                                name="xh")
                c0 = half * 4
                step = 4 // nsplit
                for cc in range(0, 4, step):
                    (eng or nc.sync).dma_start(
                        xh[:, cc:cc + step, :],
                        xt[j, :, c0 + cc:c0 + cc + step, :])
                return xh

            # first chunk's loads split across two queues so the first
            # projection matmuls start as early as possible
            nc.scalar.dma_start(wk_sb[:, 0:1, :], wk_t[:, 0:1, :])
            nc.scalar.dma_start(wv_sb[:, 0:1, :], wv_t[:, 0:1, :])
            xt_pre = [load_xt_half(0, 0, nsplit=4, eng=nc.scalar),
                      load_xt_half(0, 1)]
            nc.sync.dma_start(wq_sb[:, 1:8, :], wq_t[:, 1:8, :])
            nc.sync.dma_start(wk_sb[:, 1:8, :], wk_t[:, 1:8, :])
            nc.sync.dma_start(wv_sb[:, 1:8, :], wv_t[:, 1:8, :])
            nc.scalar.dma_start(rot_sb[:], rot2t[:])
            nc.scalar.dma_start(cos_sb[:], cos2[:])
            nc.scalar.dma_start(sin_sb[:], sin2[:])
            nc.scalar.dma_start(id_sb[:], ident[:])
            nc.scalar.dma_start(tri_sb[:], trimask[:])
            # wo is only needed ~150us in; keep it off the startup queues
            nc.gpsimd.dma_start(wo_sb[:], wo_t[:])

            # zero the pad halves of krope; ones block of v_all
            nc.vector.memset(krope[64:128, 0, :], 0.0)
            nc.vector.memset(krope[0:64, 1, :], 0.0)
            nc.vector.memset(v_all[:, :, :, :, 64:128], 1.0)

            # ---- phase A: projections + rope + V transposes ----
            def emit_a(j, xh01=None):
                xh = xh01 or [load_xt_half(j, 0), load_xt_half(j, 1)]
                ps_q = ppool.tile([128, 512], F32, tag="pP", bufs=3)
                ps_k = ppool.tile([128, 512], F32, tag="pP", bufs=3,
                                  name="ps_k")
                ps_v = ppool.tile([128, 512], F32, tag="pP", bufs=3,
                                  name="ps_v")
                for f in range(NF):
                    st, sp = (f == 0), (f == NF - 1)
                    src = xh[f // 4][:, f % 4, :]
                    nc.tensor.matmul(ps_q[:], wq_sb[:, f, :], src,
                                     start=st, stop=sp)
                    nc.tensor.matmul(ps_k[:], wk_sb[:, f, :], src,
                                     start=st, stop=sp)
                    nc.tensor.matmul(ps_v[:], wv_sb[:, f, :], src,
                                     start=st, stop=sp)
                qT = wpool.tile([128, 512], BF16, tag="qT", bufs=1)
                kT = wpool.tile([128, 512], BF16, tag="kT", bufs=1)
                vT = wpool.tile([128, 512], BF16, tag="vT", bufs=2)
                nc.vector.tensor_copy(qT[:], ps_q[:])
                nc.vector.tensor_copy(kT[:], ps_k[:])
                nc.vector.tensor_copy(vT[:], ps_v[:])
                # rotation matmuls (engine ops cannot cross partitions)
                ps_rq = ppool.tile([128, 512], F32, tag="pB", bufs=2,
                                   name="ps_rq")
                nc.tensor.matmul(ps_rq[:], rot_sb[:], qT[:],
                                 start=True, stop=True)
                ps_rk = ppool.tile([128, 512], F32, tag="pB", bufs=2,
                                   name="ps_rk")
                nc.tensor.matmul(ps_rk[:], rot_sb[:], kT[:],
                                 start=True, stop=True)
                tl = (j % 4) * 512
                J = slice(j * 512, (j + 1) * 512)
                TL = slice(tl, tl + 512)
                tmp = wpool.tile([128, 512], BF16, tag="ropetmp", bufs=2,
                                 name="tmp")
                nc.vector.tensor_mul(tmp[:], ps_rq[:], sin_sb[:, TL])
                nc.vector.tensor_mul(qrope[:, J], qT[:], cos_sb[:, TL])
                nc.vector.tensor_add(qrope[:, J], qrope[:, J], tmp[:])
                tmpk = wpool.tile([128, 512], BF16, tag="ropetmp", bufs=2,
                                  name="tmpk")
                nc.vector.tensor_mul(tmpk[:], ps_rk[:], sin_sb[:, TL])
                for h in range(HPC):
                    hs = slice(h * 64, (h + 1) * 64)
                    nc.vector.tensor_mul(krope[hs, h, J], kT[hs, :],
                                         cos_sb[hs, TL])
                    nc.vector.tensor_add(krope[hs, h, J], krope[hs, h, J],
                                         tmpk[hs, :])
                return (j, vT)

            def v_transposes(j, vT):
                b = j // 4
                for h in range(HPC):
                    hs = slice(h * 64, (h + 1) * 64)
                    ps_t = ppool.tile([128, 4, 64], BF16, tag="pB",
                                      bufs=2, name="ps_t")
                    for tt in range(4):
                        nc.tensor.transpose(
                            ps_t[:, tt, :],
                            vT[hs, tt * 128:(tt + 1) * 128],
                            id_sb[hs, hs],
                        )
                    nc.vector.tensor_copy(
                        v_all[:, b, h, (j % 4) * 4:(j % 4) * 4 + 4, 0:64],
                        ps_t[:])

            # ---- phase B: attention ----
            def scores_mm(h, base, q0, kt, n_full):
                k0 = kt * 128
                v = kt - n_full
                ps_s = ppool.tile([128, 512], F32, tag="pS", bufs=3,
                                  name="ps_s")
                if v < 0:
                    nc.tensor.matmul(
                        ps_s[:],
                        krope[:, h, base + k0:base + k0 + 128],
                        qrope[:, base + q0:base + q0 + 512],
                        start=True, stop=True,
                    )
                else:
                    # diagonal tile: queries below q0 + 128v are fully
                    # masked (skip those columns); the within-block causal
                    # mask is a -60 bias added by a second matmul
                    # (tri_sb^T @ I) so no engine touches ae afterwards
                    nc.tensor.matmul(
                        ps_s[:, v * 128:512],
                        krope[:, h, base + k0:base + k0 + 128],
                        qrope[:, base + q0 + v * 128:base + q0 + 512],
                        start=True, stop=False, skip_group_check=True,
                    )
                    nc.tensor.matmul(
                        ps_s[:, v * 128:(v + 1) * 128],
                        tri_sb[:], id_sb[:],
                        start=False, stop=True, skip_group_check=True,
                    )
                return ps_s

            def exp_mask(ps_s, n_full, kt):
                ae = wpool.tile([128, 512], BF16, tag="attexp", bufs=3,
                                name="ae")
                if kt < n_full:
                    nc.scalar.activation(
                        ae[:], ps_s[:], mybir.ActivationFunctionType.Exp)
                else:
                    v = kt - n_full
                    nc.scalar.activation(
                        ae[:, v * 128:512], ps_s[:, v * 128:512],
                        mybir.ActivationFunctionType.Exp)
                return ae

            def norm_piece(attv, h, b, qc, p0, p1, on_dve=False):
                """Normalize + stage query columns [128*p0, 128*p1) of a
                chunk: rowsums (PSUM rows 64:128, replicated by the ones
                block) go ScalarE->SBUF (same partitions), DMA-shift to
                partitions 0:64, reciprocal, multiply-cast, stage."""
                q0 = qc * QCHUNK
                w = (p1 - p0) * 128
                cs = slice(p0 * 128, p1 * 128)
                sums_hi = wpool.tile([128, 512], F32, tag="sums_hi", bufs=3)
                if on_dve:
                    # the tail chunk's chain avoids ScalarE (long exp queue)
                    nc.vector.tensor_copy(sums_hi[64:128, cs],
                                          attv[64:128, cs])
                else:
                    nc.scalar.copy(sums_hi[64:128, cs], attv[64:128, cs])
                sums = wpool.tile([64, 512], F32, tag="sums", bufs=3)
                nc.scalar.dma_start(sums[:, 0:w], sums_hi[64:128, cs])
                nc.vector.reciprocal(sums[:, 0:w], sums[:, 0:w])
                nc.vector.tensor_mul(
                    att_out[:, b, h, q0 + p0 * 128:q0 + p1 * 128],
                    attv[0:64, cs], sums[:, 0:w])
                hf = qc // 2
                c0 = 4 * (qc % 2)
                nc.sync.dma_start(
                    a2a_in[b][hf][c0 + p0:c0 + p1, 64 * h:64 * h + 64, :]
                    .transpose([1, 0, 2]),
                    att_out[:, b, h, q0 + p0 * 128:q0 + p1 * 128]
                    .rearrange("p (c q) -> p c q", c=p1 - p0),
                )

            def emit_b(h, b, qc, piecewise=False):
                base = b * T
                q0 = qc * QCHUNK
                n_full = q0 // 128
                n_kt = n_full + 4
                attv = ppool.tile([128, 512], F32, tag="pB", bufs=2)
                PIPE = 3
                pend_s = [scores_mm(h, base, q0, kt, n_full)
                          for kt in range(min(PIPE, n_kt))]
                for kt in range(n_kt):
                    ae = exp_mask(pend_s[kt], n_full, kt)
                    if kt + PIPE < n_kt:
                        pend_s.append(scores_mm(h, base, q0, kt + PIPE, n_full))
                    # diagonal tiles only contribute to query columns
                    # >= their own block: slice instead of zero-padding ae
                    v = max(kt - n_full, 0)
                    nc.tensor.matmul(
                        attv[:, v * 128:512],
                        v_all[:, b, h, kt, :],
                        ae[:, v * 128:512],
                        start=(kt == 0), stop=(kt == n_kt - 1),
                        skip_group_check=True,
                    )
                    if piecewise and kt >= n_full:
                        # column block v is final once its diagonal tile
                        # lands: normalize + stage it immediately so the
                        # A2A trigger chain after the last matmul is short
                        v = kt - n_full
                        norm_piece(attv, h, b, qc, v, v + 1, on_dve=True)
                if not piecewise:
                    norm_piece(attv, h, b, qc, 0, 4)

            def emit_a2a(a_in, a_out, att_dst):
                nc.gpsimd.collective_compute(
                    "AllToAll", mybir.AluOpType.bypass,
                    replica_groups=[list(range(N_CORES))],
                    ins=[a_in.opt()],
                    outs=[a_out.opt()],
                )
                nc.gpsimd.dma_start(
                    att_dst[:], a_out[:].transpose([1, 0, 2]))

            def proj(b, s):
                """Output rows for my batch-b token tile s (K=128)."""
                for oc in range(2):
                    ps_o = ppool.tile([128, 512], F32, tag="pP", bufs=3,
                                      name="ps_o")
                    for c in range(N_CORES):
                        nc.tensor.matmul(
                            ps_o[:],
                            att_alls[b][s][:, c, :],
                            wo_sb[:, c, oc * 512:(oc + 1) * 512],
                            start=(c == 0), stop=(c == N_CORES - 1),
                        )
                    o_sb = wpool.tile([128, 512], F32, tag="osb", bufs=2)
                    nc.vector.tensor_copy(o_sb[:], ps_o[:])
                    nc.sync.dma_start(
                        out[b * 256 + s * 128:b * 256 + (s + 1) * 128,
                            oc * 512:(oc + 1) * 512],
                        o_sb[:],
                    )

            # ---- schedule ----
            pend = emit_a(0, xt_pre)
            for j in range(1, 5):
                nxt = emit_a(j)
                v_transposes(*pend)
                pend = nxt
            # b0 attention interleaved with phase A chunks 5-7 (b1)
            emit_b(0, 0, 0)
            nxt = emit_a(5)
            v_transposes(*pend)
            pend = nxt
            emit_b(1, 0, 0)
            nxt = emit_a(6)
            v_transposes(*pend)
            pend = nxt
            emit_b(0, 0, 1)
            nxt = emit_a(7)
            v_transposes(*pend)
            pend = nxt
            emit_b(1, 0, 1)
            v_transposes(*pend)  # batch 1 V complete
            emit_a2a(a2a_in[0][0], a2a_out[0][0], att_alls[0][0])
            emit_b(0, 0, 2)
            emit_b(1, 0, 2)
            emit_b(0, 0, 3)
            emit_b(1, 0, 3)
            emit_a2a(a2a_in[0][1], a2a_out[0][1], att_alls[0][1])
            # b1 attention; b0 projections weave in once A2As have landed
            emit_b(0, 1, 0)
            emit_b(1, 1, 0)
            emit_b(0, 1, 1)
            emit_b(1, 1, 1)
            emit_a2a(a2a_in[1][0], a2a_out[1][0], att_alls[1][0])
            emit_b(0, 1, 2)
            emit_b(1, 1, 2)
            emit_b(0, 1, 3)
            emit_b(1, 1, 3, piecewise=True)
            # all ready projections at the tail: they overlap the last
            # A2A, and the scheduler may hoist them into late attention
            # bubbles. The tile_wait_until gates (sim-time, scheduler-
            # only) stop the hoist from going so early that a proj waits
            # on a collective that hasn't landed on real hardware (the
            # first A2A takes ~35us on hw vs ~5us in the scheduler's
            # model). Emitted BEFORE the last collective so they don't
            # pick up a coarsened same-queue wait on its landing DMA.
            with tc.tile_wait_until(0.135):
                proj(0, 0)
            with tc.tile_wait_until(0.155):
                proj(0, 1)
            with tc.tile_wait_until(0.175):
                proj(1, 0)
            emit_a2a(a2a_in[1][1], a2a_out[1][1], att_alls[1][1])
            proj(1, 1)

            if debug:
                nc.sync.dma_start(dbg_qrope[:], qrope[:].bitcast(F32))
                nc.sync.dma_start(
                    dbg_krope[:],
                    krope[:].rearrange("p a b -> p (a b)").bitcast(F32))
                nc.sync.dma_start(
                    dbg_vall[:],
                    v_all[:].rearrange("p a b c d -> p (a b c d)")
                    .bitcast(F32))
                nc.gpsimd.dma_start(
                    dbg_attout[:],
                    att_out[:].rearrange("p a b c -> p (a b c)").bitcast(F32))
                for b in range(B):
                    for hf in range(2):
                        off = (b * 2 + hf) * N_CORES * 128
                        nc.gpsimd.dma_start(
                            dbg_attall[:, off:off + N_CORES * 128],
                            att_alls[b][hf][:]
                            .rearrange("p a b -> p (a b)").bitcast(F32))
    nc.compile()
    return nc


def _prep_in_maps(x, wq, wk, wv, wo, cos, sin, mask):
    import ml_dtypes
    BF = ml_dtypes.bfloat16
    # xt[j, p, c, t] = x[j*512 + t, c*128 + p]
    xt = np.ascontiguousarray(
        x.reshape(NTC, 512, NF, 128).transpose(0, 3, 2, 1)).astype(BF)
    # wo_t[p, c, o] = wo[o, c*128 + p]
    wo_t = np.ascontiguousarray(
        wo.T.reshape(NF, 128, D).transpose(1, 0, 2)).astype(BF)
    cos2 = np.ascontiguousarray(np.tile(cos.T, (HPC, 1))).astype(BF)
    sin2 = np.ascontiguousarray(np.tile(sin.T, (HPC, 1))).astype(BF)
    rot2t = np.ascontiguousarray(_rot_matrix().T).astype(BF)
    ident = np.eye(128, dtype=np.float32).astype(BF)
    # trimask is a bias matrix added to diagonal score tiles via
    # ps_s += trimask^T @ I: ps_s[k, q] += trimask[q, k], so masked
    # entries (k > q) get -60 -> exp ~ 1e-26 ~ 0. Derived from `mask`
    # (tril): disallowed where mask[q, k] is False.
    m128 = np.asarray(mask[0, 0, :128, :128], dtype=bool)
    trimask = np.ascontiguousarray(
        np.where(m128, 0.0, -60.0).astype(np.float32)).astype(BF)
    scale = HD ** -0.5
    in_maps = []
    for c in range(N_CORES):
        rows = slice(c * 128, (c + 1) * 128)
        in_maps.append({
            "xt": xt,
            "wq_t": np.ascontiguousarray(
                (wq[rows, :] * scale).T.reshape(NF, 128, 128)
                .transpose(1, 0, 2)).astype(BF),
            "wk_t": np.ascontiguousarray(
                wk[rows, :].T.reshape(NF, 128, 128)
                .transpose(1, 0, 2)).astype(BF),
            "wv_t": np.ascontiguousarray(
                wv[rows, :].T.reshape(NF, 128, 128)
                .transpose(1, 0, 2)).astype(BF),
            "wo_t": wo_t,
            "cos2": cos2,
            "sin2": sin2,
            "rot2t": rot2t,
            "ident": ident,
            "trimask": trimask,
        })
    return in_maps


def kernel(x, wq, wk, wv, wo, cos, sin, mask, _trace=False, _debug=False):
    x, wq, wk, wv, wo = (np.asarray(a, dtype=np.float32)
                         for a in (x, wq, wk, wv, wo))
    cos, sin = np.asarray(cos, dtype=np.float32), np.asarray(sin, dtype=np.float32)
    mask = np.asarray(mask)
    key = ("nc", _debug)
    if key not in _CACHE:
        _CACHE[key] = build(debug=_debug)
    nc = _CACHE[key]
    in_maps = _prep_in_maps(x, wq, wk, wv, wo, cos, sin, mask)
    res = bass_utils.run_bass_kernel_spmd(
        nc, in_maps, core_ids=list(range(N_CORES)), trace=_trace)
    _CACHE["last_result"] = res
    # core c's out rows: [b(2), s(2), 128] <-> global (b, 1024*s + 128*c + i)
    full = np.zeros((B, T, D), dtype=np.float32)
    for c in range(N_CORES):
        o = res.results[c]["out"].reshape(B, 2, 128, D)
        for s in range(2):
            full[:, 1024 * s + 128 * c:1024 * s + 128 * c + 128, :] = o[:, s]
    return full.astype(np.float32)


# revision 67
# speedup vs baseline: 1.1477x; 1.1477x over previous
"""Multi-head causal attention with RoPE on 8 TRN2 NeuronCores.

Problem: B=2, T=2048, D=1024, H=16 heads, head_dim=64.
  out = softmax(mask(rope(x@Wq.T) @ rope(x@Wk.T).T / 8)) @ (x@Wv.T) @ Wo.T

Sharding: tensor-parallel over heads. Core c owns heads {2c, 2c+1} and
computes Q/K/V + attention for them over all 4096 tokens; four small
AllToAlls (one per (batch, token-half)) redistribute attention outputs
to token-sharded layout; row-parallel Wo with K=128 contraction tiles.
Core c's output rows are the interleaved 128-token tiles
{t : (t//128) % 8 == c} of each batch (re-assembled on the host).

Everything runs in bf16 (fp32 PSUM accumulation): same 1 cycle/row on
the PE as f32r but half the HBM/SBUF traffic and half the DVE cost.
RoPE uses a block-diagonal rotation matmul (engine ops cannot cross
partitions) + cos/sin elementwise on DVE. K stays 128 for every matmul
(scores use per-head zero-padded rope(K)); diagonal score tiles are
column-sliced to skip fully-masked query blocks. Softmax normalization
is batched per (batch, half): rowsums are staged by ScalarE on their
own partition, DMA'd to partition 0 in one batch, inverted as
exp(-ln(s)) on ScalarE, partition-broadcast on GpSimd, applied by DVE.
"""
import sys

sys.path.insert(0, "/opt/trn_rl_repo")

import numpy as np

from concourse import bacc, mybir, tile
from concourse import bass_utils

N_CORES = 8
B, T, D, H = 2, 2048, 1024, 16
HD = D // H              # 64
HPC = H // N_CORES       # 2 heads per core
BT = B * T               # 4096
NF = D // 128            # 8 feature chunks
NTC = BT // 512          # 8 t-chunks of 512
QCHUNK = 512

F32 = mybir.dt.float32
BF16 = mybir.dt.bfloat16

_CACHE = {}


def _rot_matrix():
    """R2 = blockdiag(R, R), R@u = rotate_half(u) per 64-dim head."""
    half = HD // 2
    R = np.zeros((HD, HD), dtype=np.float32)
    for i in range(half):
        R[i, i + half] = -1.0
        R[i + half, i] = 1.0
    R2 = np.zeros((2 * HD, 2 * HD), dtype=np.float32)
    R2[:HD, :HD] = R
    R2[HD:, HD:] = R
    return R2


def build(debug=False):
    nc = bacc.Bacc("TRN2", target_bir_lowering=False, debug=False,
                   num_devices=N_CORES)

    # ---- DRAM parameters (per-core shards, host-prepped layouts) ----
    xt = nc.declare_dram_parameter("xt", [NTC, 128, NF, 512], BF16, isOutput=False)
    wq_t = nc.declare_dram_parameter("wq_t", [128, NF, 128], BF16, isOutput=False)
    wk_t = nc.declare_dram_parameter("wk_t", [128, NF, 128], BF16, isOutput=False)
    wv_t = nc.declare_dram_parameter("wv_t", [128, NF, 128], BF16, isOutput=False)
    wo_t = nc.declare_dram_parameter("wo_t", [128, NF, D], BF16, isOutput=False)
    cos2 = nc.declare_dram_parameter("cos2", [128, T], BF16, isOutput=False)
    sin2 = nc.declare_dram_parameter("sin2", [128, T], BF16, isOutput=False)
    rot2t = nc.declare_dram_parameter("rot2t", [128, 128], BF16, isOutput=False)
    ident = nc.declare_dram_parameter("ident", [128, 128], BF16, isOutput=False)
    trimask = nc.declare_dram_parameter("trimask", [128, 128], BF16, isOutput=False)
    out = nc.declare_dram_parameter("out", [512, D], F32, isOutput=True)
    if debug:
        dbg_qrope = nc.declare_dram_parameter("dbg_qrope", [128, BT], F32, isOutput=True)
        dbg_krope = nc.declare_dram_parameter("dbg_krope", [128, HPC * BT], F32, isOutput=True)
        dbg_vall = nc.declare_dram_parameter("dbg_vall", [128, B * HPC * 16 * 65], F32, isOutput=True)
        dbg_attout = nc.declare_dram_parameter("dbg_attout", [64, B * HPC * T], F32, isOutput=True)
        dbg_attall = nc.declare_dram_parameter("dbg_attall", [128, B * N_CORES * 256], F32, isOutput=True)

    with tile.TileContext(nc) as tc, nc.allow_low_precision(reason="bf16 compute"):
        with (
            tc.tile_pool(name="consts", bufs=1) as cpool,
            tc.tile_pool(name="work", bufs=1) as wpool,
            tc.tile_pool(name="psum", bufs=1, space="PSUM") as ppool,
            tc.tile_pool(name="dram", bufs=1, space="DRAM") as dpool,
        ):
            # ---- persistent tensors ----
            rot_sb = cpool.tile([128, 128], BF16, tag="rot")
            id_sb = cpool.tile([128, 128], BF16, tag="ident")
            tri_sb = cpool.tile([128, 128], BF16, tag="tri")
            cos_sb = cpool.tile([128, T], BF16, tag="cos")
            sin_sb = cpool.tile([128, T], BF16, tag="sin")
            wo_sb = cpool.tile([128, NF, D], BF16, tag="wo")
            qrope = cpool.tile([128, BT], BF16, tag="qrope")
            # K rope, zero-padded per head so scores matmuls keep K=128.
            krope = cpool.tile([128, HPC, BT], BF16, tag="krope")
            # V per (b, h): [128 t-part, 16 t-tiles, 128]; cols 64:128 are
            # all-ones so the attv matmul emits the softmax rowsum
            # replicated across PSUM partitions 64:128 for free
            v_all = cpool.tile([128, B, HPC, T // 128, 128], BF16, tag="v_all")
            # attention out: head on the free dim (engine ops can't move
            # partitions); the A2A staging DMA interleaves the two heads
            # into the payload's 128 partitions
            att_out = cpool.tile([64, B, HPC, T], BF16, tag="att_out")
            # post-A2A: all 16 heads (rows (src, 2-head pair)) x my tokens;
            # one tile per (b, half) so proj(b, s) can't pick up a false
            # dependency on a later A2A's landing DMA
            att_alls = [[cpool.tile([128, N_CORES, 128], BF16,
                                    tag=f"att_all{b}{hf}",
                                    name=f"att_all{b}{hf}")
                         for hf in range(2)] for b in range(B)]

            wq_sb = cpool.tile([128, NF, 128], BF16, tag="wq")
            wk_sb = cpool.tile([128, NF, 128], BF16, tag="wk")
            wv_sb = cpool.tile([128, NF, 128], BF16, tag="wv")



            a2a_in = [[dpool.tile([N_CORES, 128, 128], BF16,
                                  tag=f"a2a_in{b}{hf}", name=f"a2a_in{b}{hf}")
                       for hf in range(2)] for b in range(B)]
            a2a_out = [[dpool.tile([N_CORES, 128, 128], BF16,
                                   tag=f"a2a_out{b}{hf}", name=f"a2a_out{b}{hf}")
                        for hf in range(2)] for b in range(B)]

            # ---- DMA loads; the first projection matmul needs only wq
            # f-chunk 0 + xt chunk-0 f-chunk 0, so those go first.
            nc.sync.dma_start(wq_sb[:, 0:1, :], wq_t[:, 0:1, :])

            def load_xt_half(j, half, nsplit=1, eng=None):
                """One 512-token, 4-feature-chunk half of x^T (contiguous)."""
                xh = wpool.tile([128, NF // 2, 512], BF16, tag="xt", bufs=3,
                                name="xh")
                c0 = half * 4
                step = 4 // nsplit
                for cc in range(0, 4, step):
                    (eng or nc.sync).dma_start(
                        xh[:, cc:cc + step, :],
                        xt[j, :, c0 + cc:c0 + cc + step, :])
                return xh

            # first chunk's loads split across two queues so the first
            # projection matmuls start as early as possible
            nc.scalar.dma_start(wk_sb[:, 0:1, :], wk_t[:, 0:1, :])
            nc.scalar.dma_start(wv_sb[:, 0:1, :], wv_t[:, 0:1, :])
            xt_pre = [load_xt_half(0, 0, nsplit=4, eng=nc.scalar),
                      load_xt_half(0, 1)]
            nc.sync.dma_start(wq_sb[:, 1:8, :], wq_t[:, 1:8, :])
            nc.sync.dma_start(wk_sb[:, 1:8, :], wk_t[:, 1:8, :])
            nc.sync.dma_start(wv_sb[:, 1:8, :], wv_t[:, 1:8, :])
            nc.scalar.dma_start(rot_sb[:], rot2t[:])
            nc.scalar.dma_start(cos_sb[:], cos2[:])
            nc.scalar.dma_start(sin_sb[:], sin2[:])
            nc.scalar.dma_start(id_sb[:], ident[:])
            nc.scalar.dma_start(tri_sb[:], trimask[:])
            # wo is only needed ~150us in; keep it off the startup queues
            nc.gpsimd.dma_start(wo_sb[:], wo_t[:])

            # zero the pad halves of krope; ones block of v_all
            nc.vector.memset(krope[64:128, 0, :], 0.0)
            nc.vector.memset(krope[0:64, 1, :], 0.0)
            nc.vector.memset(v_all[:, :, :, :, 64:128], 1.0)

            # ---- phase A: projections + rope + V transposes ----
            def emit_a(j, xh01=None):
                xh = xh01 or [load_xt_half(j, 0), load_xt_half(j, 1)]
                ps_q = ppool.tile([128, 512], F32, tag="pP", bufs=3)
                ps_k = ppool.tile([128, 512], F32, tag="pP", bufs=3,
                                  name="ps_k")
                ps_v = ppool.tile([128, 512], F32, tag="pP", bufs=3,
                                  name="ps_v")
                for f in range(NF):
                    st, sp = (f == 0), (f == NF - 1)
                    src = xh[f // 4][:, f % 4, :]
                    nc.tensor.matmul(ps_q[:], wq_sb[:, f, :], src,
                                     start=st, stop=sp)
                    nc.tensor.matmul(ps_k[:], wk_sb[:, f, :], src,
                                     start=st, stop=sp)
                    nc.tensor.matmul(ps_v[:], wv_sb[:, f, :], src,
                                     start=st, stop=sp)
                qT = wpool.tile([128, 512], BF16, tag="qT", bufs=1)
                kT = wpool.tile([128, 512], BF16, tag="kT", bufs=1)
                vT = wpool.tile([128, 512], BF16, tag="vT", bufs=2)
                nc.vector.tensor_copy(qT[:], ps_q[:])
                nc.vector.tensor_copy(kT[:], ps_k[:])
                nc.vector.tensor_copy(vT[:], ps_v[:])
                # rotation matmuls (engine ops cannot cross partitions)
                ps_rq = ppool.tile([128, 512], F32, tag="pB", bufs=2,
                                   name="ps_rq")
                nc.tensor.matmul(ps_rq[:], rot_sb[:], qT[:],
                                 start=True, stop=True)
                ps_rk = ppool.tile([128, 512], F32, tag="pB", bufs=2,
                                   name="ps_rk")
                nc.tensor.matmul(ps_rk[:], rot_sb[:], kT[:],
                                 start=True, stop=True)
                tl = (j % 4) * 512
                J = slice(j * 512, (j + 1) * 512)
                TL = slice(tl, tl + 512)
                tmp = wpool.tile([128, 512], BF16, tag="ropetmp", bufs=2,
                                 name="tmp")
                nc.vector.tensor_mul(tmp[:], ps_rq[:], sin_sb[:, TL])
                nc.vector.tensor_mul(qrope[:, J], qT[:], cos_sb[:, TL])
                nc.vector.tensor_add(qrope[:, J], qrope[:, J], tmp[:])
                tmpk = wpool.tile([128, 512], BF16, tag="ropetmp", bufs=2,
                                  name="tmpk")
                nc.vector.tensor_mul(tmpk[:], ps_rk[:], sin_sb[:, TL])
                for h in range(HPC):
                    hs = slice(h * 64, (h + 1) * 64)
                    nc.vector.tensor_mul(krope[hs, h, J], kT[hs, :],
                                         cos_sb[hs, TL])
                    nc.vector.tensor_add(krope[hs, h, J], krope[hs, h, J],
                                         tmpk[hs, :])
                return (j, vT)

            def v_transposes(j, vT):
                b = j // 4
                for h in range(HPC):
                    hs = slice(h * 64, (h + 1) * 64)
                    ps_t = ppool.tile([128, 4, 64], BF16, tag="pB",
                                      bufs=2, name="ps_t")
                    for tt in range(4):
                        nc.tensor.transpose(
                            ps_t[:, tt, :],
                            vT[hs, tt * 128:(tt + 1) * 128],
                            id_sb[hs, hs],
                        )
                    nc.vector.tensor_copy(
                        v_all[:, b, h, (j % 4) * 4:(j % 4) * 4 + 4, 0:64],
                        ps_t[:])

            # ---- phase B: attention ----
            def scores_mm(h, base, q0, kt, n_full):
                k0 = kt * 128
                v = kt - n_full
                ps_s = ppool.tile([128, 512], F32, tag="pS", bufs=3,
                                  name="ps_s")
                if v < 0:
                    nc.tensor.matmul(
                        ps_s[:],
                        krope[:, h, base + k0:base + k0 + 128],
                        qrope[:, base + q0:base + q0 + 512],
                        start=True, stop=True,
                    )
                else:
                    # diagonal tile: queries below q0 + 128v are fully
                    # masked (skip those columns); the within-block causal
                    # mask is a -60 bias added by a second matmul
                    # (tri_sb^T @ I) so no engine touches ae afterwards
                    nc.tensor.matmul(
                        ps_s[:, v * 128:512],
                        krope[:, h, base + k0:base + k0 + 128],
                        qrope[:, base + q0 + v * 128:base + q0 + 512],
                        start=True, stop=False, skip_group_check=True,
                    )
                    nc.tensor.matmul(
                        ps_s[:, v * 128:(v + 1) * 128],
                        tri_sb[:], id_sb[:],
                        start=False, stop=True, skip_group_check=True,
                    )
                return ps_s

            def exp_mask(ps_s, n_full, kt):
                ae = wpool.tile([128, 512], BF16, tag="attexp", bufs=3,
                                name="ae")
                if kt < n_full:
                    nc.scalar.activation(
                        ae[:], ps_s[:], mybir.ActivationFunctionType.Exp)
                else:
                    v = kt - n_full
                    nc.scalar.activation(
                        ae[:, v * 128:512], ps_s[:, v * 128:512],
                        mybir.ActivationFunctionType.Exp)
                return ae

            def norm_piece(attv, h, b, qc, p0, p1, on_dve=False):
                """Normalize + stage query columns [128*p0, 128*p1) of a
                chunk. The attv PSUM tile is released by two quick copies
                (unnormalized cast + rowsum park) so the next-next chunk
                never waits on the slow reciprocal; the normalization
                multiply then runs in-place on att_out."""
                q0 = qc * QCHUNK
                w = (p1 - p0) * 128
                cs = slice(p0 * 128, p1 * 128)
                oslc = slice(q0 + p0 * 128, q0 + p1 * 128)
                nc.vector.tensor_copy(att_out[:, b, h, oslc], attv[0:64, cs])
                sums_hi = wpool.tile([128, 512], F32, tag="sums_hi", bufs=3)
                if on_dve:
                    # the tail chunk's chain avoids ScalarE (long exp queue)
                    nc.vector.tensor_copy(sums_hi[64:128, cs],
                                          attv[64:128, cs])
                else:
                    nc.scalar.copy(sums_hi[64:128, cs], attv[64:128, cs])
                sums = wpool.tile([64, 512], F32, tag="sums", bufs=3)
                nc.scalar.dma_start(sums[:, 0:w], sums_hi[64:128, cs])
                nc.vector.reciprocal(sums[:, 0:w], sums[:, 0:w])
                nc.vector.tensor_mul(
                    att_out[:, b, h, oslc], att_out[:, b, h, oslc],
                    sums[:, 0:w])
                hf = qc // 2
                c0 = 4 * (qc % 2)
                nc.sync.dma_start(
                    a2a_in[b][hf][c0 + p0:c0 + p1, 64 * h:64 * h + 64, :]
                    .transpose([1, 0, 2]),
                    att_out[:, b, h, oslc]
                    .rearrange("p (c q) -> p c q", c=p1 - p0),
                )

            def emit_b(h, b, qc, piecewise=False):
                base = b * T
                q0 = qc * QCHUNK
                n_full = q0 // 128
                n_kt = n_full + 4
                attv = ppool.tile([128, 512], F32, tag="pB", bufs=2)
                PIPE = 3
                pend_s = [scores_mm(h, base, q0, kt, n_full)
                          for kt in range(min(PIPE, n_kt))]
                for kt in range(n_kt):
                    ae = exp_mask(pend_s[kt], n_full, kt)
                    if kt + PIPE < n_kt:
                        pend_s.append(scores_mm(h, base, q0, kt + PIPE, n_full))
                    # diagonal tiles only contribute to query columns
                    # >= their own block: slice instead of zero-padding ae
                    v = max(kt - n_full, 0)
                    nc.tensor.matmul(
                        attv[:, v * 128:512],
                        v_all[:, b, h, kt, :],
                        ae[:, v * 128:512],
                        start=(kt == 0), stop=(kt == n_kt - 1),
                        skip_group_check=True,
                    )
                    if piecewise and kt >= n_full:
                        # column block v is final once its diagonal tile
                        # lands: normalize + stage it immediately so the
                        # A2A trigger chain after the last matmul is short
                        v = kt - n_full
                        norm_piece(attv, h, b, qc, v, v + 1, on_dve=True)
                if not piecewise:
                    norm_piece(attv, h, b, qc, 0, 4)

            def emit_a2a(a_in, a_out, att_dst):
                nc.gpsimd.collective_compute(
                    "AllToAll", mybir.AluOpType.bypass,
                    replica_groups=[list(range(N_CORES))],
                    ins=[a_in.opt()],
                    outs=[a_out.opt()],
                )
                nc.gpsimd.dma_start(
                    att_dst[:], a_out[:].transpose([1, 0, 2]))

            def proj(b, s):
                """Output rows for my batch-b token tile s (K=128)."""
                for oc in range(2):
                    ps_o = ppool.tile([128, 512], F32, tag="pP", bufs=3,
                                      name="ps_o")
                    for c in range(N_CORES):
                        nc.tensor.matmul(
                            ps_o[:],
                            att_alls[b][s][:, c, :],
                            wo_sb[:, c, oc * 512:(oc + 1) * 512],
                            start=(c == 0), stop=(c == N_CORES - 1),
                        )
                    o_sb = wpool.tile([128, 512], F32, tag="osb", bufs=2)
                    nc.vector.tensor_copy(o_sb[:], ps_o[:])
                    nc.sync.dma_start(
                        out[b * 256 + s * 128:b * 256 + (s + 1) * 128,
                            oc * 512:(oc + 1) * 512],
                        o_sb[:],
                    )

            # ---- schedule ----
            pend = emit_a(0, xt_pre)
            for j in range(1, 5):
                nxt = emit_a(j)
                v_transposes(*pend)
                pend = nxt
            # b0 attention interleaved with phase A chunks 5-7 (b1)
            emit_b(0, 0, 0)
            nxt = emit_a(5)
            v_transposes(*pend)
            pend = nxt
            emit_b(1, 0, 0)
            nxt = emit_a(6)
            v_transposes(*pend)
            pend = nxt
            emit_b(0, 0, 1)
            nxt = emit_a(7)
            v_transposes(*pend)
            pend = nxt
            emit_b(1, 0, 1)
            v_transposes(*pend)  # batch 1 V complete
            emit_a2a(a2a_in[0][0], a2a_out[0][0], att_alls[0][0])
            emit_b(0, 0, 2)
            emit_b(1, 0, 2)
            emit_b(0, 0, 3)
            emit_b(1, 0, 3)
            emit_a2a(a2a_in[0][1], a2a_out[0][1], att_alls[0][1])
            # b1 attention; b0 projections weave in once A2As have landed
            emit_b(0, 1, 0)
            emit_b(1, 1, 0)
            emit_b(0, 1, 1)
            emit_b(1, 1, 1)
            emit_a2a(a2a_in[1][0], a2a_out[1][0], att_alls[1][0])
            emit_b(0, 1, 2)
            emit_b(1, 1, 2)
            emit_b(0, 1, 3)
            emit_b(1, 1, 3, piecewise=True)
            # all ready projections at the tail: they overlap the last
            # A2A, and the scheduler may hoist them into late attention
            # bubbles. The tile_wait_until gates (sim-time, scheduler-
            # only) stop the hoist from going so early that a proj waits
            # on a collective that hasn't landed on real hardware (the
            # first A2A takes ~35us on hw vs ~5us in the scheduler's
            # model). Emitted BEFORE the last collective so they don't
            # pick up a coarsened same-queue wait on its landing DMA.
            with tc.tile_wait_until(0.135):
                proj(0, 0)
            with tc.tile_wait_until(0.155):
                proj(0, 1)
            with tc.tile_wait_until(0.175):
                proj(1, 0)
            emit_a2a(a2a_in[1][1], a2a_out[1][1], att_alls[1][1])
            proj(1, 1)

            if debug:
                nc.sync.dma_start(dbg_qrope[:], qrope[:].bitcast(F32))
                nc.sync.dma_start(
                    dbg_krope[:],
                    krope[:].rearrange("p a b -> p (a b)").bitcast(F32))
                nc.sync.dma_start(
                    dbg_vall[:],
                    v_all[:].rearrange("p a b c d -> p (a b c d)")
                    .bitcast(F32))
                nc.gpsimd.dma_start(
                    dbg_attout[:],
                    att_out[:].rearrange("p a b c -> p (a b c)").bitcast(F32))
                for b in range(B):
                    for hf in range(2):
                        off = (b * 2 + hf) * N_CORES * 128
                        nc.gpsimd.dma_start(
                            dbg_attall[:, off:off + N_CORES * 128],
                            att_alls[b][hf][:]
                            .rearrange("p a b -> p (a b)").bitcast(F32))
    nc.compile()
    return nc


def _prep_in_maps(x, wq, wk, wv, wo, cos, sin, mask):
    import ml_dtypes
    BF = ml_dtypes.bfloat16
    # xt[j, p, c, t] = x[j*512 + t, c*128 + p]
    xt = np.ascontiguousarray(
        x.reshape(NTC, 512, NF, 128).transpose(0, 3, 2, 1)).astype(BF)
    # wo_t[p, c, o] = wo[o, c*128 + p]
    wo_t = np.ascontiguousarray(
        wo.T.reshape(NF, 128, D).transpose(1, 0, 2)).astype(BF)
    cos2 = np.ascontiguousarray(np.tile(cos.T, (HPC, 1))).astype(BF)
    sin2 = np.ascontiguousarray(np.tile(sin.T, (HPC, 1))).astype(BF)
    rot2t = np.ascontiguousarray(_rot_matrix().T).astype(BF)
    ident = np.eye(128, dtype=np.float32).astype(BF)
    # trimask is a bias matrix added to diagonal score tiles via
    # ps_s += trimask^T @ I: ps_s[k, q] += trimask[q, k], so masked
    # entries (k > q) get -60 -> exp ~ 1e-26 ~ 0. Derived from `mask`
    # (tril): disallowed where mask[q, k] is False.
    m128 = np.asarray(mask[0, 0, :128, :128], dtype=bool)
    trimask = np.ascontiguousarray(
        np.where(m128, 0.0, -60.0).astype(np.float32)).astype(BF)
    scale = HD ** -0.5
    in_maps = []
    for c in range(N_CORES):
        rows = slice(c * 128, (c + 1) * 128)
        in_maps.append({
            "xt": xt,
            "wq_t": np.ascontiguousarray(
                (wq[rows, :] * scale).T.reshape(NF, 128, 128)
                .transpose(1, 0, 2)).astype(BF),
            "wk_t": np.ascontiguousarray(
                wk[rows, :].T.reshape(NF, 128, 128)
                .transpose(1, 0, 2)).astype(BF),
            "wv_t": np.ascontiguousarray(
                wv[rows, :].T.reshape(NF, 128, 128)
                .transpose(1, 0, 2)).astype(BF),
            "wo_t": wo_t,
            "cos2": cos2,
            "sin2": sin2,
            "rot2t": rot2t,
            "ident": ident,
            "trimask": trimask,
        })
    return in_maps


def kernel(x, wq, wk, wv, wo, cos, sin, mask, _trace=False, _debug=False):
    x, wq, wk, wv, wo = (np.asarray(a, dtype=np.float32)
                         for a in (x, wq, wk, wv, wo))
    cos, sin = np.asarray(cos, dtype=np.float32), np.asarray(sin, dtype=np.float32)
    mask = np.asarray(mask)
    key = ("nc", _debug)
    if key not in _CACHE:
        _CACHE[key] = build(debug=_debug)
    nc = _CACHE[key]
    in_maps = _prep_in_maps(x, wq, wk, wv, wo, cos, sin, mask)
    res = bass_utils.run_bass_kernel_spmd(
        nc, in_maps, core_ids=list(range(N_CORES)), trace=_trace)
    _CACHE["last_result"] = res
    # core c's out rows: [b(2), s(2), 128] <-> global (b, 1024*s + 128*c + i)
    full = np.zeros((B, T, D), dtype=np.float32)
    for c in range(N_CORES):
        o = res.results[c]["out"].reshape(B, 2, 128, D)
        for s in range(2):
            full[:, 1024 * s + 128 * c:1024 * s + 128 * c + 128, :] = o[:, s]
    return full.astype(np.float32)
